# revision 25
# baseline (speedup 1.0000x reference)
"""Trainium2 Bass kernel for nn_MDCN (mixture-density head forward), v4.

Reference (B=2048, F=1024, M=128):
    rho = tanh(feature @ h2rho_w.T + h2rho_b);  rho[:, 0] = 0.95
    pi  = softmax(feature @ h2pi_w.T + h2pi_b)
    var0 = exp(feature @ h2var_w.T + h2var_b)
    var = (1 - exp(rho)) * var0 + 1e-4
    W_ = r*muW + s*(r*(zstd/wstd)*(W-muW) + Z*s),  s = sqrt(1-r^2)
    mu = einsum('bmf,bf->bm', W_, feature)

Algebra: with a = (zstd/wstd)*(W-muW),
    mu[b,m] = r*d1[b] + r*s*d2[b] + s^2*d3[b],
    d1 = feature@muW, d2 = feature@a, d3 = feature@Z,
so everything is ONE fused matmul per (chunk, batch-tile):
    feature @ [ -wrho.T | muW a Z 0 | wpi.T | wvar.T ]
with s = (1+tanh(u))*exp(-u), and tau=1e-4 dropped (8e-6 of max|var|).

v4 changes vs v3 (driven by loop-decomposition microbenchmarks;
18.65us -> 16.2us):
 - v3 had ~zero DMA/matmul overlap (matmul phase added its full +6.2us
   serial cold time after the stream).  v4 orders the psum group so data
   matmuls ride the chunk DMAs as their sems fire (bias matmuls END the
   group; accumulation is order-independent), and warms the PE with
   filler matmuls during the DMA window (HAM clock gate: idle PE runs
   1.2 GHz, 476ns/matmul with LDW unhidden; ~3.4us of sustained filler
   activity trips it to 2.4 GHz).  Matmul phase now adds only ~2us.
 - All three outputs go out via prepared SWDGE kv_writebacks fired by
   ~25ns triggers in completion order (PI_EARLY: pi, var, mu — the pi
   exp is scheduled 2nd on ACT so softmax normalization fires first),
   so each transfer's HBM receipt overlaps the remaining epilogue.
   Keep epilogue ALU off gpsimd: Q7 is busy generating the three
   writeback descriptor sets (VAR_ON_POOL=True costs +10.7us).
 - Measured walls (8 cores, loop-diff method): input stream 1.29MB is
   hard-serialized at ~175GB/s/core (~7.5us) regardless of queue count,
   HWDGE/SWDGE mix, or separate tiles/tensors; loop+tiny-tail overhead
   ~3us.  fp8 inputs would halve the stream but fail the 2e-2 rel-err
   gate (best variant 2.6e-2, measured via host simulation).
 - A/B'd: bias-FIRST regresses +2.3us (PE gates on the bias SWDGE DMA,
   whose completion queues behind the input stream); a 1-chunk last DMA
   regresses (lone small DMAs have pathological completion latency);
   equal 4x2-chunk split is best of {8x1, 4x2, 2x4, 1x8, 3221}.
"""

import time
from contextlib import ExitStack

import numpy as np

import concourse.bass as bass
import concourse.bacc as bacc
import concourse.mybir as mybir
import concourse.tile as tile
from concourse.bass_utils import run_bass_kernel_spmd

B, F, M = 2048, 1024, 128
NCORES = 8
BC = B // NCORES            # 256 rows/core
KC = F // 128               # 8 contraction chunks
NW = 3 * M + 4              # 388 fused psum cols: [-u | d1 d2 d3 pad | pi | var]
RHO_1 = np.float32(0.95)
S0 = float(np.sqrt(np.float32(1.0) - RHO_1 * RHO_1))

F32 = mybir.dt.float32
F16 = mybir.dt.float16
AF = mybir.ActivationFunctionType
OP = mybir.AluOpType
AX = mybir.AxisListType

MM_NP = np.float16

# psum column layout
C_U0, C_U1 = 0, M               # -u (negated rho logits)
C_D0, C_D1 = M, M + 4           # d1 d2 d3 pad
C_P0, C_P1 = M + 4, 2 * M + 4   # pi logits
C_V0, C_V1 = 2 * M + 4, 3 * M + 4  # var logits

# --- tuning flags ---
USE_TRIGGER = True        # pi|var out via prepared SWDGE writeback + trigger
VAR_ON_POOL = False
ACT_ORDER_Q_EARLY = True
STOP_AFTER = None         # None | "dma" | "mm" diagnostics
DMA_SPLITS = (2, 2, 2, 2)  # chunks per input DMA (A/B'd best: 4 x 330KB)
N_FILLERS = 7             # PE warm-up matmuls during the DMA window
FILLER_COLS = 512
BIAS_LAST = True          # bias matmuls end the psum group (A/B'd best:
                          # bias-first gates the PE on the SWDGE bias DMA,
                          # whose completion queues behind the input stream)
SPLIT_PV = True           # fire var writeback early, pi writeback late
MU_TRIGGER = True         # mu via third prepared writeback instead of HWDGE
PI_EARLY = True           # pi exp right after tanh on ACT; pi fires first
OUT_HWDGE = False         # outputs via plain HWDGE DMAs (no Q7 desc-gen)
PI_FP8 = True             # pi-head weights in e3m4 (saves 128KB of stream;
                          # pi rel-err ~1e-2 vs the 2e-2 gate)
PI_SCALE = 256.0          # power-of-2 upscale into e3m4 range; 512
                          # overflows to inf for |w| near 1/32
                          # (e3m4 max normal 15.5)


def _emit_body(nc, tc, pools, dram, warm):
    consts, blkpool, psum, work = pools
    blk_d, bias_d, outmu_d, outvar_d, outpi_d, wpi8_d = dram

    # bias block on the SWDGE queue (keeps the two HWDGE queues clear);
    # only needed at the END of the matmul phase now.
    bias = consts.tile([1, 128 + NW], F16, tag="bias", name="bias")
    nc.gpsimd.dma_start(bias[:], bias_d)

    out_v = out_p = out_mu = None
    if STOP_AFTER is None:
        out_v = work.tile([128, 1, 2, M], F16, tag="out_v", name="out_v")
        out_p = work.tile([128, 1, 2, M], F16, tag="out_p", name="out_p")
        out_mu = work.tile([128, 1, 2, M], F16, tag="out_mu", name="out_mu")
    if USE_TRIGGER and not OUT_HWDGE and STOP_AFTER is None:
        ctx0 = consts.tile([128, 2], mybir.dt.int32, tag="ctx0", name="ctx0")
        nc.vector.memset(ctx0[:], 0)
        pv_sem = nc.alloc_semaphore("pv_dma")
        # two prepared writebacks: var fires early (its receipt overlaps
        # the pi chain), pi fires at the end
        # prep order == trigger firing order (ring FIFO)
        if PI_EARLY:
            nc.gpsimd.kv_writeback(outpi_d, out_p[:], ctx0[:],
                                   prepare_only=True, sem=pv_sem)
            nc.gpsimd.kv_writeback(outvar_d, out_v[:], ctx0[:],
                                   prepare_only=True, sem=pv_sem)
            if MU_TRIGGER:
                nc.gpsimd.kv_writeback(outmu_d, out_mu[:], ctx0[:],
                                       prepare_only=True, sem=pv_sem)
        else:
            if MU_TRIGGER:
                nc.gpsimd.kv_writeback(outmu_d, out_mu[:], ctx0[:],
                                       prepare_only=True, sem=pv_sem)
            nc.gpsimd.kv_writeback(outvar_d, out_v[:], ctx0[:],
                                   prepare_only=True, sem=pv_sem)
            nc.gpsimd.kv_writeback(outpi_d, out_p[:], ctx0[:],
                                   prepare_only=True, sem=pv_sem)

    # input stream: equal DMAs alternating the two HWDGE queues; with
    # PI_FP8 each chunk row ends with M f16 cols holding the e3m4
    # pi|var weights (read back via bitcast)
    nwm = (M + 4) if PI_FP8 else NW
    ncols = 2 * 128 + nwm + (M if PI_FP8 else 0)
    blk = blkpool.tile([128, KC, ncols], F16, tag="blk", name="blk")
    pos = 0
    for i, w in enumerate(DMA_SPLITS):
        q = nc.sync if i % 2 == 0 else nc.scalar
        q.dma_start(blk[:, pos:pos + w, :], blk_d[:, pos:pos + w, :])
        pos += w
    assert pos == KC

    if STOP_AFTER == "dma":
        o = work.tile([1, 16], F16, tag="tiny", name="tiny")
        nc.vector.tensor_copy(o[:], blk[0:1, KC - 1, 0:16])
        nc.sync.dma_start(outmu_d[0, 0:1, 0:16], o[:])
        return

    # PE warm-up fillers: sustained matmul activity during the DMA wait
    # trips the HAM clock gate to 2.4 GHz before the real matmuls.
    P = psum.tile([128, 2, 512], F32, tag="P", name="P")
    P2 = None
    if PI_FP8:
        # pi matmuls get their own banks: offset-0 writes, clean group
        P2 = psum.tile([128, 2, 512], F32, tag="P2", name="P2")
    if N_FILLERS:
        scratch = psum.tile([128, FILLER_COLS], F32, tag="pe_scr", name="pe_scr")
        for _ in range(N_FILLERS):
            nc.tensor.matmul(scratch[:], warm[:, 0:128], warm[:, 0:FILLER_COLS],
                             start=True, stop=True)

    # data matmuls in chunk order; bias placement via BIAS_LAST flag
    if BIAS_LAST:
        for c in range(KC):
            for t in range(2):
                nc.tensor.matmul(P[:, t, 0:nwm],
                                 blk[:, c, t * 128:(t + 1) * 128],
                                 blk[:, c, 256:256 + nwm],
                                 start=(c == 0), stop=False)
                if PI_FP8:
                    w8 = blk[:, c, 256 + nwm:256 + nwm + M]
                    nc.tensor.matmul(P2[:, t, 0:2 * M],
                                     blk[:, c, t * 128:(t + 1) * 128],
                                     w8.bitcast(mybir.dt.float8e3),
                                     start=(c == 0), stop=False)
        for t in range(2):
            nc.tensor.matmul(P[:, t, 0:nwm], bias[:, 0:128],
                             bias[:, 128:128 + nwm], start=False, stop=True)
            if PI_FP8:
                nc.tensor.matmul(P2[:, t, 0:2 * M], bias[:, 0:128],
                                 bias[:, 128 + nwm:128 + nwm + 2 * M],
                                 start=False, stop=True)
    else:
        for t in range(2):
            nc.tensor.matmul(P[:, t, 0:NW], bias[:, 0:128],
                             bias[:, 128:128 + NW], start=True, stop=False)
        for c in range(KC):
            for t in range(2):
                nc.tensor.matmul(P[:, t, 0:NW],
                                 blk[:, c, t * 128:(t + 1) * 128],
                                 blk[:, c, 256:256 + NW],
                                 start=False, stop=(c == KC - 1))

    if STOP_AFTER == "mm":
        o = work.tile([1, 16], F32, tag="tiny", name="tiny")
        nc.vector.tensor_copy(o[:], P[0:1, 0, 0:16])
        o2 = work.tile([1, 16], F16, tag="tiny2", name="tiny2")
        nc.vector.tensor_copy(o2[:], o[:])
        nc.sync.dma_start(outmu_d[0, 0:1, 0:16], o2[:])
        return

    if PI_EARLY:
        _epilogue_pi_early(nc, pools, P, P2, out_v, out_p, out_mu,
                           outmu_d, outvar_d, outpi_d)
        return

    # ---- epilogue (fused [128, 2, .] over both batch tiles) ----
    r = work.tile([128, 2, M], F32, tag="r", name="r")
    nc.scalar.activation(r[:], P[:, :, C_U0:C_U1], AF.Tanh, scale=-1.0)
    eneg = work.tile([128, 2, M], F32, tag="eneg", name="eneg")
    nc.scalar.activation(eneg[:], P[:, :, C_U0:C_U1], AF.Exp)

    dsb = work.tile([128, 2, 3], F32, tag="dsb", name="dsb")
    nc.vector.tensor_copy(dsb[:], P[:, :, C_D0:C_D0 + 3])

    # erho from the UNCLAMPED r; z column 0 patched to the constant 1-e^0.95
    erho = work.tile([128, 2, M], F32, tag="erho", name="erho")
    E2 = work.tile([128, 2, 2 * M], F32, tag="E2", name="E2")
    q = work.tile([128, 2, M], F32, tag="q", name="q")
    s = work.tile([128, 2, M], F32, tag="s", name="s")

    nc.scalar.activation(erho[:], r[:], AF.Exp)
    if not ACT_ORDER_Q_EARLY:
        nc.scalar.activation(E2[:], P[:, :, C_P0:C_V1], AF.Exp)

    zeng = nc.gpsimd if VAR_ON_POOL else nc.vector
    z = work.tile([128, 2, M], F32, tag="z", name="z")
    zeng.tensor_scalar(z[:], erho[:], -1.0, 1.0, OP.mult, OP.add)
    for t in range(2):
        zeng.memset(z[:, t, 0:1], float(1.0 - np.exp(RHO_1)))
        nc.vector.memset(r[:, t, 0:1], float(RHO_1))

    nc.vector.scalar_tensor_tensor(s[:], r[:], 1.0, eneg[:], OP.add, OP.mult)
    for t in range(2):
        nc.vector.memset(s[:, t, 0:1], S0)
    ss = work.tile([128, 2, M], F32, tag="ss", name="ss")
    nc.vector.tensor_tensor(ss[:], s[:], s[:], OP.mult)

    for t in range(2):
        nc.scalar.activation(q[:, t, :], s[:, t, :], AF.Identity,
                             bias=dsb[:, t, 0:1], scale=dsb[:, t, 1:2])
    if ACT_ORDER_Q_EARLY:
        nc.scalar.activation(E2[:], P[:, :, C_P0:C_V1], AF.Exp)

    if not MU_TRIGGER:
        out_mu = work.tile([128, 2, M], F16, tag="out_mu", name="out_mu")
    rq = work.tile([128, 2, M], F32, tag="rq", name="rq")
    for t in range(2):
        nc.vector.tensor_tensor(rq[:, t, :], r[:, t, :], q[:, t, :], OP.mult)
        mu_dst = out_mu[:, 0, t, :] if MU_TRIGGER else out_mu[:, t, :]
        nc.vector.scalar_tensor_tensor(mu_dst, ss[:, t, :],
                                       dsb[:, t, 2:3], rq[:, t, :],
                                       OP.mult, OP.add)
    if MU_TRIGGER:
        nc.gpsimd.trigger_dma(count=1, signals_writable=[out_mu[:]])
    else:
        nc.sync.dma_start(outmu_d.rearrange("t p o j -> p (o t) j"),
                          out_mu[:])

    epi, var0 = E2[:, :, 0:M], E2[:, :, M:2 * M]
    zeng.tensor_tensor(out_v[:, 0, :, :], z[:], var0, OP.mult)
    if USE_TRIGGER and SPLIT_PV:
        nc.gpsimd.trigger_dma(count=1, signals_writable=[out_v[:]])
    ssum = work.tile([128, 2, 1], F32, tag="ssum", name="ssum")
    nc.vector.tensor_reduce(ssum[:], epi, AX.X, OP.add)
    rsum = work.tile([128, 2, 1], F32, tag="rsum", name="rsum")
    nc.vector.reciprocal(rsum[:], ssum[:])
    for t in range(2):
        nc.vector.tensor_scalar_mul(out_p[:, 0, t, :], E2[:, t, 0:M],
                                    rsum[:, t, 0:1])
    if USE_TRIGGER:
        if SPLIT_PV:
            nc.gpsimd.trigger_dma(count=1, signals_writable=[out_p[:]])
        else:
            nc.gpsimd.trigger_dma(count=None,
                                  signals_writable=[out_v[:], out_p[:]])
    else:
        nc.sync.dma_start(outvar_d.rearrange("t p o j -> p o t j"), out_v[:])
        nc.sync.dma_start(outpi_d.rearrange("t p o j -> p o t j"), out_p[:])


def _epilogue_pi_early(nc, pools, P, P2, out_v, out_p, out_mu,
                       outmu_d, outvar_d, outpi_d):
    """Epilogue variant: pi exp scheduled 2nd on ACT, pi writeback fires
    first; var then mu follow in expected completion order."""
    consts, blkpool, psum, work = pools

    if PI_FP8:
        pi_src, var_src = P2[:, :, 0:M], P2[:, :, M:2 * M]
        pi_scale = var_scale = 1.0 / PI_SCALE
    else:
        pi_src, var_src = P[:, :, C_P0:C_P1], P[:, :, C_V0:C_V1]
        pi_scale = var_scale = 1.0
    r = work.tile([128, 2, M], F32, tag="r", name="r")
    nc.scalar.activation(r[:], P[:, :, C_U0:C_U1], AF.Tanh, scale=-1.0)
    epi_t = work.tile([128, 2, M], F32, tag="epi", name="epi")
    nc.scalar.activation(epi_t[:], pi_src, AF.Exp, scale=pi_scale)
    eneg = work.tile([128, 2, M], F32, tag="eneg", name="eneg")
    nc.scalar.activation(eneg[:], P[:, :, C_U0:C_U1], AF.Exp)
    erho = work.tile([128, 2, M], F32, tag="erho", name="erho")
    nc.scalar.activation(erho[:], r[:], AF.Exp)
    evar = work.tile([128, 2, M], F32, tag="evar", name="evar")
    nc.scalar.activation(evar[:], var_src, AF.Exp, scale=var_scale)

    dsb = work.tile([128, 2, 3], F32, tag="dsb", name="dsb")
    nc.vector.tensor_copy(dsb[:], P[:, :, C_D0:C_D0 + 3])

    # pi chain first on DVE
    ssum = work.tile([128, 2, 1], F32, tag="ssum", name="ssum")
    nc.vector.tensor_reduce(ssum[:], epi_t[:], AX.X, OP.add)
    rsum = work.tile([128, 2, 1], F32, tag="rsum", name="rsum")
    nc.vector.reciprocal(rsum[:], ssum[:])
    for t in range(2):
        nc.vector.tensor_scalar_mul(out_p[:, 0, t, :], epi_t[:, t, :],
                                    rsum[:, t, 0:1])
    if OUT_HWDGE:
        nc.sync.dma_start(outpi_d.rearrange("t p o j -> p o t j"), out_p[:])
    else:
        nc.gpsimd.trigger_dma(count=1, signals_writable=[out_p[:]])

    # var chain
    z = work.tile([128, 2, M], F32, tag="z", name="z")
    nc.vector.tensor_scalar(z[:], erho[:], -1.0, 1.0, OP.mult, OP.add)
    for t in range(2):
        nc.vector.memset(z[:, t, 0:1], float(1.0 - np.exp(RHO_1)))
        nc.vector.memset(r[:, t, 0:1], float(RHO_1))
    nc.vector.tensor_tensor(out_v[:, 0, :, :], z[:], evar[:], OP.mult)
    if OUT_HWDGE:
        nc.scalar.dma_start(outvar_d.rearrange("t p o j -> p o t j"), out_v[:])
    else:
        nc.gpsimd.trigger_dma(count=1, signals_writable=[out_v[:]])

    # mu chain
    s = work.tile([128, 2, M], F32, tag="s", name="s")
    nc.vector.scalar_tensor_tensor(s[:], r[:], 1.0, eneg[:], OP.add, OP.mult)
    for t in range(2):
        nc.vector.memset(s[:, t, 0:1], S0)
    ss = work.tile([128, 2, M], F32, tag="ss", name="ss")
    nc.vector.tensor_tensor(ss[:], s[:], s[:], OP.mult)
    q = work.tile([128, 2, M], F32, tag="q", name="q")
    for t in range(2):
        nc.scalar.activation(q[:, t, :], s[:, t, :], AF.Identity,
                             bias=dsb[:, t, 0:1], scale=dsb[:, t, 1:2])
    rq = work.tile([128, 2, M], F32, tag="rq", name="rq")
    for t in range(2):
        nc.vector.tensor_tensor(rq[:, t, :], r[:, t, :], q[:, t, :], OP.mult)
        nc.vector.scalar_tensor_tensor(out_mu[:, 0, t, :], ss[:, t, :],
                                       dsb[:, t, 2:3], rq[:, t, :],
                                       OP.mult, OP.add)
    if OUT_HWDGE:
        nc.sync.dma_start(outmu_d.rearrange("t p o j -> p o t j"), out_mu[:])
    else:
        nc.gpsimd.trigger_dma(count=1, signals_writable=[out_mu[:]])


def _declare_io(nc):
    # with PI_FP8 the last M f16 cols are the e3m4 pi|var weights (bitcast)
    ncols = (2 * 128 + (M + 4) + M) if PI_FP8 else (2 * 128 + NW)
    blk_d = nc.dram_tensor("blk", [128, KC, ncols], F16,
                           kind="ExternalInput").ap()
    wpi8_d = None
    bias_d = nc.dram_tensor("bias", [1, 128 + NW], F16,
                            kind="ExternalInput").ap()
    outmu_d = nc.dram_tensor("out_mu", [2, 128, 1, M], F16,
                             kind="ExternalOutput").ap()
    outvar_d = nc.dram_tensor("out_var", [2, 128, 1, M], F16,
                              kind="ExternalOutput").ap()
    outpi_d = nc.dram_tensor("out_pi", [2, 128, 1, M], F16,
                             kind="ExternalOutput").ap()
    return blk_d, bias_d, outmu_d, outvar_d, outpi_d, wpi8_d


def _warmup_act(nc, consts):
    # Load the Tanh/Exp ACT table immediately (one-time ~1.3us), and
    # init the PE filler source tile.
    warm_in = consts.tile([128, 1], F32, tag="warm_in", name="warm_in")
    warm_out = consts.tile([128, 1], F32, tag="warm_out", name="warm_out")
    nc.vector.memset(warm_in[:], 0.0)
    nc.scalar.activation(warm_out[:], warm_in[:], AF.Exp)
    warm = consts.tile([128, FILLER_COLS], F16, tag="pe_w", name="pe_w")
    nc.vector.memset(warm[:], 0.001)
    return warm


def _build(reps=None):
    nc = bacc.Bacc("TRN2", target_bir_lowering=False, debug=False)
    dram = _declare_io(nc)
    with tile.TileContext(nc) as tc, ExitStack() as ctx:
        consts = ctx.enter_context(tc.tile_pool(name="consts", bufs=1))
        blkpool = ctx.enter_context(tc.tile_pool(name="blk", bufs=1))
        psum = ctx.enter_context(tc.tile_pool(name="psum", bufs=1, space="PSUM"))
        work = ctx.enter_context(tc.tile_pool(name="work", bufs=1))
        pools = (consts, blkpool, psum, work)
        assert PI_EARLY or not PI_FP8, "PI_FP8 requires the PI_EARLY epilogue"
        warm = _warmup_act(nc, consts)
        if reps is None:
            _emit_body(nc, tc, pools, dram, warm)
        else:
            with tc.For_i(0, reps, 1):
                _emit_body(nc, tc, pools, dram, warm)
    nc.compile()
    return nc


def build_loop_nc(reps):
    return _build(reps=reps)


_CACHE = {}


def _get_nc():
    if "nc" not in _CACHE:
        _CACHE["nc"] = _build()
    return _CACHE["nc"]


def _host_prep(inputs):
    f32 = np.float32
    feature = np.ascontiguousarray(inputs["feature"], dtype=f32)
    muW = np.asarray(inputs["muW"], dtype=f32)
    W = np.asarray(inputs["W"], dtype=f32)
    Z = np.asarray(inputs["Z"], dtype=f32)
    logvarW = np.asarray(inputs["logvarW"], dtype=f32)
    logvarZ = np.asarray(inputs["logvarZ"], dtype=f32)

    wstd = np.sqrt(np.exp(logvarW)).astype(f32)
    zstd = np.sqrt(np.exp(logvarZ)).astype(f32)
    a = ((zstd / wstd).astype(f32) * (W - muW)).astype(f32)
    v3 = np.stack([muW, a, Z, np.zeros_like(muW)], axis=1)      # [F, 4]

    wpi8 = None
    if PI_FP8:
        import ml_dtypes
        wcat = np.concatenate(
            [-np.asarray(inputs["h2rho_w"], dtype=f32).T, v3],
            axis=1)                                                # [F, 132]
        wcat = wcat.reshape(KC, 128, M + 4).astype(MM_NP)
        w8 = np.concatenate(
            [np.asarray(inputs["h2pi_w"], dtype=f32).T,
             np.asarray(inputs["h2var_w"], dtype=f32).T],
            axis=1) * PI_SCALE                                     # [F, 256]
        wpi8 = np.ascontiguousarray(
            w8.astype(ml_dtypes.float8_e3m4)
            .reshape(KC, 128, 2 * M).transpose(1, 0, 2)).view(np.uint8)
        bias = np.concatenate(
            [np.ones(128, dtype=f32),
             -np.asarray(inputs["h2rho_b"], dtype=f32),
             np.zeros(4, dtype=f32),
             np.asarray(inputs["h2pi_b"], dtype=f32) * np.float32(PI_SCALE),
             np.asarray(inputs["h2var_b"], dtype=f32) * np.float32(PI_SCALE)]
        ).reshape(1, 128 + NW).astype(MM_NP)
    else:
        wcat = np.concatenate(
            [-np.asarray(inputs["h2rho_w"], dtype=f32).T, v3,
             np.asarray(inputs["h2pi_w"], dtype=f32).T,
             np.asarray(inputs["h2var_w"], dtype=f32).T], axis=1)    # [F, 388]
        wcat = wcat.reshape(KC, 128, NW).astype(MM_NP)

        bias = np.concatenate(
            [np.ones(128, dtype=f32),
             -np.asarray(inputs["h2rho_b"], dtype=f32), np.zeros(4, dtype=f32),
             np.asarray(inputs["h2pi_b"], dtype=f32),
             np.asarray(inputs["h2var_b"], dtype=f32)]
        ).reshape(1, 128 + NW).astype(MM_NP)

    nwm = (M + 4) if PI_FP8 else NW
    ncols = 2 * 128 + nwm + (M if PI_FP8 else 0)
    in_maps = []
    for cidx in range(NCORES):
        shard = feature[cidx * BC:(cidx + 1) * BC]       # [256, F]
        featT = shard.T.reshape(KC, 128, BC).astype(MM_NP)
        blk = np.empty((128, KC, ncols), dtype=MM_NP)
        blk[:, :, 0:256] = featT.transpose(1, 0, 2)
        blk[:, :, 256:256 + nwm] = wcat.transpose(1, 0, 2)
        if PI_FP8:
            blk[:, :, 256 + nwm:] = wpi8.view(np.float16)
        m = {"blk": np.ascontiguousarray(blk), "bias": bias}
        in_maps.append(m)
    return in_maps


def kernel(**inputs):
    nc = _get_nc()
    in_maps = _host_prep(inputs)
    res = run_bass_kernel_spmd(nc, in_maps, list(range(NCORES)))
    mu = np.concatenate(
        [res.results[c]["out_mu"].reshape(BC, M) for c in range(NCORES)],
        axis=0).astype(np.float32)
    var = np.concatenate(
        [res.results[c]["out_var"].reshape(BC, M) for c in range(NCORES)],
        axis=0).astype(np.float32)
    pi = np.concatenate(
        [res.results[c]["out_pi"].reshape(BC, M) for c in range(NCORES)],
        axis=0).astype(np.float32)
    return pi, mu, var


# revision 27
# speedup vs baseline: 1.0791x; 1.0791x over previous
"""Trainium2 Bass kernel for nn_MDCN (mixture-density head forward), v4.

Reference (B=2048, F=1024, M=128):
    rho = tanh(feature @ h2rho_w.T + h2rho_b);  rho[:, 0] = 0.95
    pi  = softmax(feature @ h2pi_w.T + h2pi_b)
    var0 = exp(feature @ h2var_w.T + h2var_b)
    var = (1 - exp(rho)) * var0 + 1e-4
    W_ = r*muW + s*(r*(zstd/wstd)*(W-muW) + Z*s),  s = sqrt(1-r^2)
    mu = einsum('bmf,bf->bm', W_, feature)

Algebra: with a = (zstd/wstd)*(W-muW),
    mu[b,m] = r*d1[b] + r*s*d2[b] + s^2*d3[b],
    d1 = feature@muW, d2 = feature@a, d3 = feature@Z,
so everything is ONE fused matmul per (chunk, batch-tile):
    feature @ [ -wrho.T | muW a Z 0 | wpi.T | wvar.T ]
with s = (1+tanh(u))*exp(-u), and tau=1e-4 dropped (8e-6 of max|var|).

v4 changes vs v3 (driven by loop-decomposition microbenchmarks;
18.65us -> 16.2us):
 - v3 had ~zero DMA/matmul overlap (matmul phase added its full +6.2us
   serial cold time after the stream).  v4 orders the psum group so data
   matmuls ride the chunk DMAs as their sems fire (bias matmuls END the
   group; accumulation is order-independent), and warms the PE with
   filler matmuls during the DMA window (HAM clock gate: idle PE runs
   1.2 GHz, 476ns/matmul with LDW unhidden; ~3.4us of sustained filler
   activity trips it to 2.4 GHz).  Matmul phase now adds only ~2us.
 - All three outputs go out via prepared SWDGE kv_writebacks fired by
   ~25ns triggers in completion order (PI_EARLY: pi, var, mu — the pi
   exp is scheduled 2nd on ACT so softmax normalization fires first),
   so each transfer's HBM receipt overlaps the remaining epilogue.
   Keep epilogue ALU off gpsimd: Q7 is busy generating the three
   writeback descriptor sets (VAR_ON_POOL=True costs +10.7us).
 - Measured walls (8 cores, loop-diff method): input stream 1.29MB is
   hard-serialized at ~175GB/s/core (~7.5us) regardless of queue count,
   HWDGE/SWDGE mix, or separate tiles/tensors; loop+tiny-tail overhead
   ~3us.  fp8 inputs would halve the stream but fail the 2e-2 rel-err
   gate (best variant 2.6e-2, measured via host simulation).
 - A/B'd: bias-FIRST regresses +2.3us (PE gates on the bias SWDGE DMA,
   whose completion queues behind the input stream); a 1-chunk last DMA
   regresses (lone small DMAs have pathological completion latency);
   equal 4x2-chunk split is best of {8x1, 4x2, 2x4, 1x8, 3221}.
"""

import time
from contextlib import ExitStack

import numpy as np

import concourse.bass as bass
import concourse.bacc as bacc
import concourse.mybir as mybir
import concourse.tile as tile
from concourse.bass_utils import run_bass_kernel_spmd

B, F, M = 2048, 1024, 128
NCORES = 8
BC = B // NCORES            # 256 rows/core
KC = F // 128               # 8 contraction chunks
NW = 3 * M + 4              # 388 fused psum cols: [-u | d1 d2 d3 pad | pi | var]
RHO_1 = np.float32(0.95)
S0 = float(np.sqrt(np.float32(1.0) - RHO_1 * RHO_1))

F32 = mybir.dt.float32
F16 = mybir.dt.float16
AF = mybir.ActivationFunctionType
OP = mybir.AluOpType
AX = mybir.AxisListType

MM_NP = np.float16

# psum column layout
C_U0, C_U1 = 0, M               # -u (negated rho logits)
C_D0, C_D1 = M, M + 4           # d1 d2 d3 pad
C_P0, C_P1 = M + 4, 2 * M + 4   # pi logits
C_V0, C_V1 = 2 * M + 4, 3 * M + 4  # var logits

# --- tuning flags ---
USE_TRIGGER = True        # pi|var out via prepared SWDGE writeback + trigger
VAR_ON_POOL = False
ACT_ORDER_Q_EARLY = True
STOP_AFTER = None         # None | "dma" | "mm" diagnostics
DMA_SPLITS = (2, 2, 2, 2)  # chunks per input DMA (A/B'd best: 4 x 330KB)
N_FILLERS = 7             # PE warm-up matmuls during the DMA window
FILLER_COLS = 512
BIAS_LAST = True          # bias matmuls end the psum group (A/B'd best:
                          # bias-first gates the PE on the SWDGE bias DMA,
                          # whose completion queues behind the input stream)
SPLIT_PV = True           # fire var writeback early, pi writeback late
MU_TRIGGER = True         # mu via third prepared writeback instead of HWDGE
PI_EARLY = True           # pi exp right after tanh on ACT; pi fires first
DENSE_DRAM = True         # pack each DMA's 330KB as one dense DRAM block
                          # (sequential HBM access vs 10KB-strided rows)
OUT_HWDGE = False         # outputs via plain HWDGE DMAs (no Q7 desc-gen)


def _emit_body(nc, tc, pools, dram, warm):
    consts, blkpool, psum, work = pools
    blk_d, bias_d, outmu_d, outvar_d, outpi_d = dram

    # bias block on the SWDGE queue (keeps the two HWDGE queues clear);
    # only needed at the END of the matmul phase now.
    bias = consts.tile([1, 128 + NW], F16, tag="bias", name="bias")
    nc.gpsimd.dma_start(bias[:], bias_d)

    out_v = out_p = out_mu = None
    if STOP_AFTER is None:
        out_v = work.tile([128, 1, 2, M], F16, tag="out_v", name="out_v")
        out_p = work.tile([128, 1, 2, M], F16, tag="out_p", name="out_p")
        out_mu = work.tile([128, 1, 2, M], F16, tag="out_mu", name="out_mu")
    if USE_TRIGGER and not OUT_HWDGE and STOP_AFTER is None:
        ctx0 = consts.tile([128, 2], mybir.dt.int32, tag="ctx0", name="ctx0")
        nc.vector.memset(ctx0[:], 0)
        pv_sem = nc.alloc_semaphore("pv_dma")
        # two prepared writebacks: var fires early (its receipt overlaps
        # the pi chain), pi fires at the end
        # prep order == trigger firing order (ring FIFO)
        if PI_EARLY:
            nc.gpsimd.kv_writeback(outpi_d, out_p[:], ctx0[:],
                                   prepare_only=True, sem=pv_sem)
            nc.gpsimd.kv_writeback(outvar_d, out_v[:], ctx0[:],
                                   prepare_only=True, sem=pv_sem)
            if MU_TRIGGER:
                nc.gpsimd.kv_writeback(outmu_d, out_mu[:], ctx0[:],
                                       prepare_only=True, sem=pv_sem)
        else:
            if MU_TRIGGER:
                nc.gpsimd.kv_writeback(outmu_d, out_mu[:], ctx0[:],
                                       prepare_only=True, sem=pv_sem)
            nc.gpsimd.kv_writeback(outvar_d, out_v[:], ctx0[:],
                                   prepare_only=True, sem=pv_sem)
            nc.gpsimd.kv_writeback(outpi_d, out_p[:], ctx0[:],
                                   prepare_only=True, sem=pv_sem)

    # input stream: equal DMAs alternating the two HWDGE queues; with
    # DENSE_DRAM each DMA's source is one dense DRAM block (sequential
    # HBM access) instead of 10KB-strided partition rows
    blk = blkpool.tile([128, KC, 2 * 128 + NW], F16, tag="blk", name="blk")
    pos = 0
    for i, w in enumerate(DMA_SPLITS):
        q = nc.sync if i % 2 == 0 else nc.scalar
        if DENSE_DRAM:
            q.dma_start(blk[:, pos:pos + w, :], blk_d[i, :, :])
        else:
            q.dma_start(blk[:, pos:pos + w, :], blk_d[:, pos:pos + w, :])
        pos += w
    assert pos == KC

    if STOP_AFTER == "dma":
        o = work.tile([1, 16], F16, tag="tiny", name="tiny")
        nc.vector.tensor_copy(o[:], blk[0:1, KC - 1, 0:16])
        nc.sync.dma_start(outmu_d[0, 0:1, 0:16], o[:])
        return

    # PE warm-up fillers: sustained matmul activity during the DMA wait
    # trips the HAM clock gate to 2.4 GHz before the real matmuls.
    P = psum.tile([128, 2, 512], F32, tag="P", name="P")
    if N_FILLERS:
        scratch = psum.tile([128, FILLER_COLS], F32, tag="pe_scr", name="pe_scr")
        for _ in range(N_FILLERS):
            nc.tensor.matmul(scratch[:], warm[:, 0:128], warm[:, 0:FILLER_COLS],
                             start=True, stop=True)

    # data matmuls in chunk order; bias placement via BIAS_LAST flag
    if BIAS_LAST:
        for c in range(KC):
            for t in range(2):
                nc.tensor.matmul(P[:, t, 0:NW],
                                 blk[:, c, t * 128:(t + 1) * 128],
                                 blk[:, c, 256:256 + NW],
                                 start=(c == 0), stop=False)
        for t in range(2):
            nc.tensor.matmul(P[:, t, 0:NW], bias[:, 0:128],
                             bias[:, 128:128 + NW], start=False, stop=True)
    else:
        for t in range(2):
            nc.tensor.matmul(P[:, t, 0:NW], bias[:, 0:128],
                             bias[:, 128:128 + NW], start=True, stop=False)
        for c in range(KC):
            for t in range(2):
                nc.tensor.matmul(P[:, t, 0:NW],
                                 blk[:, c, t * 128:(t + 1) * 128],
                                 blk[:, c, 256:256 + NW],
                                 start=False, stop=(c == KC - 1))

    if STOP_AFTER == "mm":
        o = work.tile([1, 16], F32, tag="tiny", name="tiny")
        nc.vector.tensor_copy(o[:], P[0:1, 0, 0:16])
        o2 = work.tile([1, 16], F16, tag="tiny2", name="tiny2")
        nc.vector.tensor_copy(o2[:], o[:])
        nc.sync.dma_start(outmu_d[0, 0:1, 0:16], o2[:])
        return

    if PI_EARLY:
        _epilogue_pi_early(nc, pools, P, out_v, out_p, out_mu,
                           outmu_d, outvar_d, outpi_d)
        return

    # ---- epilogue (fused [128, 2, .] over both batch tiles) ----
    r = work.tile([128, 2, M], F32, tag="r", name="r")
    nc.scalar.activation(r[:], P[:, :, C_U0:C_U1], AF.Tanh, scale=-1.0)
    eneg = work.tile([128, 2, M], F32, tag="eneg", name="eneg")
    nc.scalar.activation(eneg[:], P[:, :, C_U0:C_U1], AF.Exp)

    dsb = work.tile([128, 2, 3], F32, tag="dsb", name="dsb")
    nc.vector.tensor_copy(dsb[:], P[:, :, C_D0:C_D0 + 3])

    # erho from the UNCLAMPED r; z column 0 patched to the constant 1-e^0.95
    erho = work.tile([128, 2, M], F32, tag="erho", name="erho")
    E2 = work.tile([128, 2, 2 * M], F32, tag="E2", name="E2")
    q = work.tile([128, 2, M], F32, tag="q", name="q")
    s = work.tile([128, 2, M], F32, tag="s", name="s")

    nc.scalar.activation(erho[:], r[:], AF.Exp)
    if not ACT_ORDER_Q_EARLY:
        nc.scalar.activation(E2[:], P[:, :, C_P0:C_V1], AF.Exp)

    zeng = nc.gpsimd if VAR_ON_POOL else nc.vector
    z = work.tile([128, 2, M], F32, tag="z", name="z")
    zeng.tensor_scalar(z[:], erho[:], -1.0, 1.0, OP.mult, OP.add)
    for t in range(2):
        zeng.memset(z[:, t, 0:1], float(1.0 - np.exp(RHO_1)))
        nc.vector.memset(r[:, t, 0:1], float(RHO_1))

    nc.vector.scalar_tensor_tensor(s[:], r[:], 1.0, eneg[:], OP.add, OP.mult)
    for t in range(2):
        nc.vector.memset(s[:, t, 0:1], S0)
    ss = work.tile([128, 2, M], F32, tag="ss", name="ss")
    nc.vector.tensor_tensor(ss[:], s[:], s[:], OP.mult)

    for t in range(2):
        nc.scalar.activation(q[:, t, :], s[:, t, :], AF.Identity,
                             bias=dsb[:, t, 0:1], scale=dsb[:, t, 1:2])
    if ACT_ORDER_Q_EARLY:
        nc.scalar.activation(E2[:], P[:, :, C_P0:C_V1], AF.Exp)

    if not MU_TRIGGER:
        out_mu = work.tile([128, 2, M], F16, tag="out_mu", name="out_mu")
    rq = work.tile([128, 2, M], F32, tag="rq", name="rq")
    for t in range(2):
        nc.vector.tensor_tensor(rq[:, t, :], r[:, t, :], q[:, t, :], OP.mult)
        mu_dst = out_mu[:, 0, t, :] if MU_TRIGGER else out_mu[:, t, :]
        nc.vector.scalar_tensor_tensor(mu_dst, ss[:, t, :],
                                       dsb[:, t, 2:3], rq[:, t, :],
                                       OP.mult, OP.add)
    if MU_TRIGGER:
        nc.gpsimd.trigger_dma(count=1, signals_writable=[out_mu[:]])
    else:
        nc.sync.dma_start(outmu_d.rearrange("t p o j -> p (o t) j"),
                          out_mu[:])

    epi, var0 = E2[:, :, 0:M], E2[:, :, M:2 * M]
    zeng.tensor_tensor(out_v[:, 0, :, :], z[:], var0, OP.mult)
    if USE_TRIGGER and SPLIT_PV:
        nc.gpsimd.trigger_dma(count=1, signals_writable=[out_v[:]])
    ssum = work.tile([128, 2, 1], F32, tag="ssum", name="ssum")
    nc.vector.tensor_reduce(ssum[:], epi, AX.X, OP.add)
    rsum = work.tile([128, 2, 1], F32, tag="rsum", name="rsum")
    nc.vector.reciprocal(rsum[:], ssum[:])
    for t in range(2):
        nc.vector.tensor_scalar_mul(out_p[:, 0, t, :], E2[:, t, 0:M],
                                    rsum[:, t, 0:1])
    if USE_TRIGGER:
        if SPLIT_PV:
            nc.gpsimd.trigger_dma(count=1, signals_writable=[out_p[:]])
        else:
            nc.gpsimd.trigger_dma(count=None,
                                  signals_writable=[out_v[:], out_p[:]])
    else:
        nc.sync.dma_start(outvar_d.rearrange("t p o j -> p o t j"), out_v[:])
        nc.sync.dma_start(outpi_d.rearrange("t p o j -> p o t j"), out_p[:])


def _epilogue_pi_early(nc, pools, P, out_v, out_p, out_mu,
                       outmu_d, outvar_d, outpi_d):
    """Epilogue variant: pi exp scheduled 2nd on ACT, pi writeback fires
    first; var then mu follow in expected completion order."""
    consts, blkpool, psum, work = pools

    r = work.tile([128, 2, M], F32, tag="r", name="r")
    nc.scalar.activation(r[:], P[:, :, C_U0:C_U1], AF.Tanh, scale=-1.0)
    epi_t = work.tile([128, 2, M], F32, tag="epi", name="epi")
    nc.scalar.activation(epi_t[:], P[:, :, C_P0:C_P1], AF.Exp)
    eneg = work.tile([128, 2, M], F32, tag="eneg", name="eneg")
    nc.scalar.activation(eneg[:], P[:, :, C_U0:C_U1], AF.Exp)
    erho = work.tile([128, 2, M], F32, tag="erho", name="erho")
    nc.scalar.activation(erho[:], r[:], AF.Exp)
    evar = work.tile([128, 2, M], F32, tag="evar", name="evar")
    nc.scalar.activation(evar[:], P[:, :, C_V0:C_V1], AF.Exp)

    dsb = work.tile([128, 2, 3], F32, tag="dsb", name="dsb")
    nc.vector.tensor_copy(dsb[:], P[:, :, C_D0:C_D0 + 3])

    # pi chain first on DVE
    ssum = work.tile([128, 2, 1], F32, tag="ssum", name="ssum")
    nc.vector.tensor_reduce(ssum[:], epi_t[:], AX.X, OP.add)
    rsum = work.tile([128, 2, 1], F32, tag="rsum", name="rsum")
    nc.vector.reciprocal(rsum[:], ssum[:])
    for t in range(2):
        nc.vector.tensor_scalar_mul(out_p[:, 0, t, :], epi_t[:, t, :],
                                    rsum[:, t, 0:1])
    if OUT_HWDGE:
        nc.sync.dma_start(outpi_d.rearrange("t p o j -> p o t j"), out_p[:])
    else:
        nc.gpsimd.trigger_dma(count=1, signals_writable=[out_p[:]])

    # var chain
    z = work.tile([128, 2, M], F32, tag="z", name="z")
    nc.vector.tensor_scalar(z[:], erho[:], -1.0, 1.0, OP.mult, OP.add)
    for t in range(2):
        nc.vector.memset(z[:, t, 0:1], float(1.0 - np.exp(RHO_1)))
        nc.vector.memset(r[:, t, 0:1], float(RHO_1))
    nc.vector.tensor_tensor(out_v[:, 0, :, :], z[:], evar[:], OP.mult)
    if OUT_HWDGE:
        nc.scalar.dma_start(outvar_d.rearrange("t p o j -> p o t j"), out_v[:])
    else:
        nc.gpsimd.trigger_dma(count=1, signals_writable=[out_v[:]])

    # mu chain
    s = work.tile([128, 2, M], F32, tag="s", name="s")
    nc.vector.scalar_tensor_tensor(s[:], r[:], 1.0, eneg[:], OP.add, OP.mult)
    for t in range(2):
        nc.vector.memset(s[:, t, 0:1], S0)
    ss = work.tile([128, 2, M], F32, tag="ss", name="ss")
    nc.vector.tensor_tensor(ss[:], s[:], s[:], OP.mult)
    q = work.tile([128, 2, M], F32, tag="q", name="q")
    for t in range(2):
        nc.scalar.activation(q[:, t, :], s[:, t, :], AF.Identity,
                             bias=dsb[:, t, 0:1], scale=dsb[:, t, 1:2])
    rq = work.tile([128, 2, M], F32, tag="rq", name="rq")
    for t in range(2):
        nc.vector.tensor_tensor(rq[:, t, :], r[:, t, :], q[:, t, :], OP.mult)
        nc.vector.scalar_tensor_tensor(out_mu[:, 0, t, :], ss[:, t, :],
                                       dsb[:, t, 2:3], rq[:, t, :],
                                       OP.mult, OP.add)
    if OUT_HWDGE:
        nc.sync.dma_start(outmu_d.rearrange("t p o j -> p o t j"), out_mu[:])
    else:
        nc.gpsimd.trigger_dma(count=1, signals_writable=[out_mu[:]])


def _declare_io(nc):
    if DENSE_DRAM:
        step = KC // len(DMA_SPLITS)
        blk_d = nc.dram_tensor("blk",
                               [len(DMA_SPLITS), 128, step * (2 * 128 + NW)],
                               F16, kind="ExternalInput").ap()
    else:
        blk_d = nc.dram_tensor("blk", [128, KC, 2 * 128 + NW], F16,
                               kind="ExternalInput").ap()
    bias_d = nc.dram_tensor("bias", [1, 128 + NW], F16,
                            kind="ExternalInput").ap()
    outmu_d = nc.dram_tensor("out_mu", [2, 128, 1, M], F16,
                             kind="ExternalOutput").ap()
    outvar_d = nc.dram_tensor("out_var", [2, 128, 1, M], F16,
                              kind="ExternalOutput").ap()
    outpi_d = nc.dram_tensor("out_pi", [2, 128, 1, M], F16,
                             kind="ExternalOutput").ap()
    return blk_d, bias_d, outmu_d, outvar_d, outpi_d


def _warmup_act(nc, consts):
    # Load the Tanh/Exp ACT table immediately (one-time ~1.3us), and
    # init the PE filler source tile.
    warm_in = consts.tile([128, 1], F32, tag="warm_in", name="warm_in")
    warm_out = consts.tile([128, 1], F32, tag="warm_out", name="warm_out")
    nc.vector.memset(warm_in[:], 0.0)
    nc.scalar.activation(warm_out[:], warm_in[:], AF.Exp)
    warm = consts.tile([128, FILLER_COLS], F16, tag="pe_w", name="pe_w")
    nc.vector.memset(warm[:], 0.001)
    return warm


def _build(reps=None):
    nc = bacc.Bacc("TRN2", target_bir_lowering=False, debug=False)
    dram = _declare_io(nc)
    with tile.TileContext(nc) as tc, ExitStack() as ctx:
        consts = ctx.enter_context(tc.tile_pool(name="consts", bufs=1))
        blkpool = ctx.enter_context(tc.tile_pool(name="blk", bufs=1))
        psum = ctx.enter_context(tc.tile_pool(name="psum", bufs=1, space="PSUM"))
        work = ctx.enter_context(tc.tile_pool(name="work", bufs=1))
        pools = (consts, blkpool, psum, work)
        warm = _warmup_act(nc, consts)
        if reps is None:
            _emit_body(nc, tc, pools, dram, warm)
        else:
            with tc.For_i(0, reps, 1):
                _emit_body(nc, tc, pools, dram, warm)
    nc.compile()
    return nc


def build_loop_nc(reps):
    return _build(reps=reps)


_CACHE = {}


def _get_nc():
    if "nc" not in _CACHE:
        _CACHE["nc"] = _build()
    return _CACHE["nc"]


def _host_prep(inputs):
    f32 = np.float32
    feature = np.ascontiguousarray(inputs["feature"], dtype=f32)
    muW = np.asarray(inputs["muW"], dtype=f32)
    W = np.asarray(inputs["W"], dtype=f32)
    Z = np.asarray(inputs["Z"], dtype=f32)
    logvarW = np.asarray(inputs["logvarW"], dtype=f32)
    logvarZ = np.asarray(inputs["logvarZ"], dtype=f32)

    wstd = np.sqrt(np.exp(logvarW)).astype(f32)
    zstd = np.sqrt(np.exp(logvarZ)).astype(f32)
    a = ((zstd / wstd).astype(f32) * (W - muW)).astype(f32)
    v3 = np.stack([muW, a, Z, np.zeros_like(muW)], axis=1)      # [F, 4]

    wcat = np.concatenate(
        [-np.asarray(inputs["h2rho_w"], dtype=f32).T, v3,
         np.asarray(inputs["h2pi_w"], dtype=f32).T,
         np.asarray(inputs["h2var_w"], dtype=f32).T], axis=1)    # [F, 388]
    wcat = wcat.reshape(KC, 128, NW).astype(MM_NP)

    bias = np.concatenate(
        [np.ones(128, dtype=f32),
         -np.asarray(inputs["h2rho_b"], dtype=f32), np.zeros(4, dtype=f32),
         np.asarray(inputs["h2pi_b"], dtype=f32),
         np.asarray(inputs["h2var_b"], dtype=f32)]
    ).reshape(1, 128 + NW).astype(MM_NP)

    in_maps = []
    for cidx in range(NCORES):
        shard = feature[cidx * BC:(cidx + 1) * BC]       # [256, F]
        featT = shard.T.reshape(KC, 128, BC).astype(MM_NP)
        blk = np.empty((128, KC, 2 * 128 + NW), dtype=MM_NP)
        blk[:, :, 0:256] = featT.transpose(1, 0, 2)
        blk[:, :, 256:256 + NW] = wcat.transpose(1, 0, 2)
        if DENSE_DRAM:
            step = KC // len(DMA_SPLITS)
            blk = np.stack(
                [blk[:, i * step:(i + 1) * step, :].reshape(
                    128, step * (2 * 128 + NW))
                 for i in range(len(DMA_SPLITS))])
        in_maps.append({"blk": np.ascontiguousarray(blk), "bias": bias})
    return in_maps


def kernel(**inputs):
    nc = _get_nc()
    in_maps = _host_prep(inputs)
    res = run_bass_kernel_spmd(nc, in_maps, list(range(NCORES)))
    mu = np.concatenate(
        [res.results[c]["out_mu"].reshape(BC, M) for c in range(NCORES)],
        axis=0).astype(np.float32)
    var = np.concatenate(
        [res.results[c]["out_var"].reshape(BC, M) for c in range(NCORES)],
        axis=0).astype(np.float32)
    pi = np.concatenate(
        [res.results[c]["out_pi"].reshape(BC, M) for c in range(NCORES)],
        axis=0).astype(np.float32)
    return pi, mu, var


# revision 28
# speedup vs baseline: 1.1001x; 1.0194x over previous
"""Trainium2 Bass kernel for nn_MDCN (mixture-density head forward), v4.

Reference (B=2048, F=1024, M=128):
    rho = tanh(feature @ h2rho_w.T + h2rho_b);  rho[:, 0] = 0.95
    pi  = softmax(feature @ h2pi_w.T + h2pi_b)
    var0 = exp(feature @ h2var_w.T + h2var_b)
    var = (1 - exp(rho)) * var0 + 1e-4
    W_ = r*muW + s*(r*(zstd/wstd)*(W-muW) + Z*s),  s = sqrt(1-r^2)
    mu = einsum('bmf,bf->bm', W_, feature)

Algebra: with a = (zstd/wstd)*(W-muW),
    mu[b,m] = r*d1[b] + r*s*d2[b] + s^2*d3[b],
    d1 = feature@muW, d2 = feature@a, d3 = feature@Z,
so everything is ONE fused matmul per (chunk, batch-tile):
    feature @ [ -wrho.T | muW a Z 0 | wpi.T | wvar.T ]
with s = (1+tanh(u))*exp(-u), and tau=1e-4 dropped (8e-6 of max|var|).

v4 changes vs v3 (driven by loop-decomposition microbenchmarks;
18.65us -> 16.2us):
 - v3 had ~zero DMA/matmul overlap (matmul phase added its full +6.2us
   serial cold time after the stream).  v4 orders the psum group so data
   matmuls ride the chunk DMAs as their sems fire (bias matmuls END the
   group; accumulation is order-independent), and warms the PE with
   filler matmuls during the DMA window (HAM clock gate: idle PE runs
   1.2 GHz, 476ns/matmul with LDW unhidden; ~3.4us of sustained filler
   activity trips it to 2.4 GHz).  Matmul phase now adds only ~2us.
 - All three outputs go out via prepared SWDGE kv_writebacks fired by
   ~25ns triggers in completion order (PI_EARLY: pi, var, mu — the pi
   exp is scheduled 2nd on ACT so softmax normalization fires first),
   so each transfer's HBM receipt overlaps the remaining epilogue.
   Keep epilogue ALU off gpsimd: Q7 is busy generating the three
   writeback descriptor sets (VAR_ON_POOL=True costs +10.7us).
 - Measured walls (8 cores, loop-diff method): input stream 1.29MB is
   hard-serialized at ~175GB/s/core (~7.5us) regardless of queue count,
   HWDGE/SWDGE mix, or separate tiles/tensors; loop+tiny-tail overhead
   ~3us.  fp8 inputs would halve the stream but fail the 2e-2 rel-err
   gate (best variant 2.6e-2, measured via host simulation).
 - A/B'd: bias-FIRST regresses +2.3us (PE gates on the bias SWDGE DMA,
   whose completion queues behind the input stream); a 1-chunk last DMA
   regresses (lone small DMAs have pathological completion latency);
   equal 4x2-chunk split is best of {8x1, 4x2, 2x4, 1x8, 3221}.
"""

import time
from contextlib import ExitStack

import numpy as np

import concourse.bass as bass
import concourse.bacc as bacc
import concourse.mybir as mybir
import concourse.tile as tile
from concourse.bass_utils import run_bass_kernel_spmd

B, F, M = 2048, 1024, 128
NCORES = 8
BC = B // NCORES            # 256 rows/core
KC = F // 128               # 8 contraction chunks
NW = 3 * M + 4              # 388 fused psum cols: [-u | d1 d2 d3 pad | pi | var]
RHO_1 = np.float32(0.95)
S0 = float(np.sqrt(np.float32(1.0) - RHO_1 * RHO_1))

F32 = mybir.dt.float32
F16 = mybir.dt.float16
AF = mybir.ActivationFunctionType
OP = mybir.AluOpType
AX = mybir.AxisListType

MM_NP = np.float16

# psum column layout
C_U0, C_U1 = 0, M               # -u (negated rho logits)
C_D0, C_D1 = M, M + 4           # d1 d2 d3 pad
C_P0, C_P1 = M + 4, 2 * M + 4   # pi logits
C_V0, C_V1 = 2 * M + 4, 3 * M + 4  # var logits

# --- tuning flags ---
USE_TRIGGER = True        # pi|var out via prepared SWDGE writeback + trigger
VAR_ON_POOL = False
ACT_ORDER_Q_EARLY = True
STOP_AFTER = None         # None | "dma" | "mm" diagnostics
DMA_SPLITS = (2, 2, 2, 2)  # chunks per input DMA (A/B'd best: 4 x 330KB)
N_FILLERS = 7             # PE warm-up matmuls during the DMA window
FILLER_COLS = 512
BIAS_LAST = True          # bias matmuls end the psum group (A/B'd best:
                          # bias-first gates the PE on the SWDGE bias DMA,
                          # whose completion queues behind the input stream)
SPLIT_PV = True           # fire var writeback early, pi writeback late
MU_TRIGGER = True         # mu via third prepared writeback instead of HWDGE
PI_EARLY = True           # pi exp right after tanh on ACT; pi fires first
OUT_HWDGE = False         # outputs via plain HWDGE DMAs (no Q7 desc-gen)


def _emit_body(nc, tc, pools, dram, warm):
    consts, blkpool, psum, work = pools
    blk_d, bias_d, outmu_d, outvar_d, outpi_d = dram

    # bias block on the SWDGE queue (keeps the two HWDGE queues clear);
    # only needed at the END of the matmul phase now.
    bias = consts.tile([1, 128 + NW], F16, tag="bias", name="bias")
    nc.gpsimd.dma_start(bias[:], bias_d)

    out_v = out_p = out_mu = None
    if STOP_AFTER is None:
        out_v = work.tile([128, 1, 2, M], F16, tag="out_v", name="out_v")
        out_p = work.tile([128, 1, 2, M], F16, tag="out_p", name="out_p")
        out_mu = work.tile([128, 1, 2, M], F16, tag="out_mu", name="out_mu")
    if USE_TRIGGER and not OUT_HWDGE and STOP_AFTER is None:
        ctx0 = consts.tile([128, 2], mybir.dt.int32, tag="ctx0", name="ctx0")
        nc.vector.memset(ctx0[:], 0)
        pv_sem = nc.alloc_semaphore("pv_dma")
        # two prepared writebacks: var fires early (its receipt overlaps
        # the pi chain), pi fires at the end
        # prep order == trigger firing order (ring FIFO)
        if PI_EARLY:
            nc.gpsimd.kv_writeback(outpi_d, out_p[:], ctx0[:],
                                   prepare_only=True, sem=pv_sem)
            nc.gpsimd.kv_writeback(outvar_d, out_v[:], ctx0[:],
                                   prepare_only=True, sem=pv_sem)
            if MU_TRIGGER:
                nc.gpsimd.kv_writeback(outmu_d, out_mu[:], ctx0[:],
                                       prepare_only=True, sem=pv_sem)
        else:
            if MU_TRIGGER:
                nc.gpsimd.kv_writeback(outmu_d, out_mu[:], ctx0[:],
                                       prepare_only=True, sem=pv_sem)
            nc.gpsimd.kv_writeback(outvar_d, out_v[:], ctx0[:],
                                   prepare_only=True, sem=pv_sem)
            nc.gpsimd.kv_writeback(outpi_d, out_p[:], ctx0[:],
                                   prepare_only=True, sem=pv_sem)

    # input stream: descending-size DMAs alternating the two HWDGE queues
    blk = blkpool.tile([128, KC, 2 * 128 + NW], F16, tag="blk", name="blk")
    pos = 0
    for i, w in enumerate(DMA_SPLITS):
        q = nc.sync if i % 2 == 0 else nc.scalar
        q.dma_start(blk[:, pos:pos + w, :], blk_d[:, pos:pos + w, :])
        pos += w
    assert pos == KC

    if STOP_AFTER == "dma":
        o = work.tile([1, 16], F16, tag="tiny", name="tiny")
        nc.vector.tensor_copy(o[:], blk[0:1, KC - 1, 0:16])
        nc.sync.dma_start(outmu_d[0, 0:1, 0:16], o[:])
        return

    # PE warm-up fillers: sustained matmul activity during the DMA wait
    # trips the HAM clock gate to 2.4 GHz before the real matmuls.
    P = psum.tile([128, 2, 512], F32, tag="P", name="P")
    if N_FILLERS:
        scratch = psum.tile([128, FILLER_COLS], F32, tag="pe_scr", name="pe_scr")
        for _ in range(N_FILLERS):
            nc.tensor.matmul(scratch[:], warm[:, 0:128], warm[:, 0:FILLER_COLS],
                             start=True, stop=True)

    # data matmuls in chunk order; bias placement via BIAS_LAST flag
    if BIAS_LAST:
        for c in range(KC):
            for t in range(2):
                nc.tensor.matmul(P[:, t, 0:NW],
                                 blk[:, c, t * 128:(t + 1) * 128],
                                 blk[:, c, 256:256 + NW],
                                 start=(c == 0), stop=False)
        for t in range(2):
            nc.tensor.matmul(P[:, t, 0:NW], bias[:, 0:128],
                             bias[:, 128:128 + NW], start=False, stop=True)
    else:
        for t in range(2):
            nc.tensor.matmul(P[:, t, 0:NW], bias[:, 0:128],
                             bias[:, 128:128 + NW], start=True, stop=False)
        for c in range(KC):
            for t in range(2):
                nc.tensor.matmul(P[:, t, 0:NW],
                                 blk[:, c, t * 128:(t + 1) * 128],
                                 blk[:, c, 256:256 + NW],
                                 start=False, stop=(c == KC - 1))

    if STOP_AFTER == "mm":
        o = work.tile([1, 16], F32, tag="tiny", name="tiny")
        nc.vector.tensor_copy(o[:], P[0:1, 0, 0:16])
        o2 = work.tile([1, 16], F16, tag="tiny2", name="tiny2")
        nc.vector.tensor_copy(o2[:], o[:])
        nc.sync.dma_start(outmu_d[0, 0:1, 0:16], o2[:])
        return

    if PI_EARLY:
        _epilogue_pi_early(nc, pools, P, out_v, out_p, out_mu,
                           outmu_d, outvar_d, outpi_d)
        return

    # ---- epilogue (fused [128, 2, .] over both batch tiles) ----
    r = work.tile([128, 2, M], F32, tag="r", name="r")
    nc.scalar.activation(r[:], P[:, :, C_U0:C_U1], AF.Tanh, scale=-1.0)
    eneg = work.tile([128, 2, M], F32, tag="eneg", name="eneg")
    nc.scalar.activation(eneg[:], P[:, :, C_U0:C_U1], AF.Exp)

    dsb = work.tile([128, 2, 3], F32, tag="dsb", name="dsb")
    nc.vector.tensor_copy(dsb[:], P[:, :, C_D0:C_D0 + 3])

    # erho from the UNCLAMPED r; z column 0 patched to the constant 1-e^0.95
    erho = work.tile([128, 2, M], F32, tag="erho", name="erho")
    E2 = work.tile([128, 2, 2 * M], F32, tag="E2", name="E2")
    q = work.tile([128, 2, M], F32, tag="q", name="q")
    s = work.tile([128, 2, M], F32, tag="s", name="s")

    nc.scalar.activation(erho[:], r[:], AF.Exp)
    if not ACT_ORDER_Q_EARLY:
        nc.scalar.activation(E2[:], P[:, :, C_P0:C_V1], AF.Exp)

    zeng = nc.gpsimd if VAR_ON_POOL else nc.vector
    z = work.tile([128, 2, M], F32, tag="z", name="z")
    zeng.tensor_scalar(z[:], erho[:], -1.0, 1.0, OP.mult, OP.add)
    for t in range(2):
        zeng.memset(z[:, t, 0:1], float(1.0 - np.exp(RHO_1)))
        nc.vector.memset(r[:, t, 0:1], float(RHO_1))

    nc.vector.scalar_tensor_tensor(s[:], r[:], 1.0, eneg[:], OP.add, OP.mult)
    for t in range(2):
        nc.vector.memset(s[:, t, 0:1], S0)
    ss = work.tile([128, 2, M], F32, tag="ss", name="ss")
    nc.vector.tensor_tensor(ss[:], s[:], s[:], OP.mult)

    for t in range(2):
        nc.scalar.activation(q[:, t, :], s[:, t, :], AF.Identity,
                             bias=dsb[:, t, 0:1], scale=dsb[:, t, 1:2])
    if ACT_ORDER_Q_EARLY:
        nc.scalar.activation(E2[:], P[:, :, C_P0:C_V1], AF.Exp)

    if not MU_TRIGGER:
        out_mu = work.tile([128, 2, M], F16, tag="out_mu", name="out_mu")
    rq = work.tile([128, 2, M], F32, tag="rq", name="rq")
    for t in range(2):
        nc.vector.tensor_tensor(rq[:, t, :], r[:, t, :], q[:, t, :], OP.mult)
        mu_dst = out_mu[:, 0, t, :] if MU_TRIGGER else out_mu[:, t, :]
        nc.vector.scalar_tensor_tensor(mu_dst, ss[:, t, :],
                                       dsb[:, t, 2:3], rq[:, t, :],
                                       OP.mult, OP.add)
    if MU_TRIGGER:
        nc.gpsimd.trigger_dma(count=1, signals_writable=[out_mu[:]])
    else:
        nc.sync.dma_start(outmu_d.rearrange("t p o j -> p (o t) j"),
                          out_mu[:])

    epi, var0 = E2[:, :, 0:M], E2[:, :, M:2 * M]
    zeng.tensor_tensor(out_v[:, 0, :, :], z[:], var0, OP.mult)
    if USE_TRIGGER and SPLIT_PV:
        nc.gpsimd.trigger_dma(count=1, signals_writable=[out_v[:]])
    ssum = work.tile([128, 2, 1], F32, tag="ssum", name="ssum")
    nc.vector.tensor_reduce(ssum[:], epi, AX.X, OP.add)
    rsum = work.tile([128, 2, 1], F32, tag="rsum", name="rsum")
    nc.vector.reciprocal(rsum[:], ssum[:])
    for t in range(2):
        nc.vector.tensor_scalar_mul(out_p[:, 0, t, :], E2[:, t, 0:M],
                                    rsum[:, t, 0:1])
    if USE_TRIGGER:
        if SPLIT_PV:
            nc.gpsimd.trigger_dma(count=1, signals_writable=[out_p[:]])
        else:
            nc.gpsimd.trigger_dma(count=None,
                                  signals_writable=[out_v[:], out_p[:]])
    else:
        nc.sync.dma_start(outvar_d.rearrange("t p o j -> p o t j"), out_v[:])
        nc.sync.dma_start(outpi_d.rearrange("t p o j -> p o t j"), out_p[:])


def _epilogue_pi_early(nc, pools, P, out_v, out_p, out_mu,
                       outmu_d, outvar_d, outpi_d):
    """Epilogue variant: pi exp scheduled 2nd on ACT, pi writeback fires
    first; var then mu follow in expected completion order."""
    consts, blkpool, psum, work = pools

    r = work.tile([128, 2, M], F32, tag="r", name="r")
    nc.scalar.activation(r[:], P[:, :, C_U0:C_U1], AF.Tanh, scale=-1.0)
    epi_t = work.tile([128, 2, M], F32, tag="epi", name="epi")
    nc.scalar.activation(epi_t[:], P[:, :, C_P0:C_P1], AF.Exp)
    eneg = work.tile([128, 2, M], F32, tag="eneg", name="eneg")
    nc.scalar.activation(eneg[:], P[:, :, C_U0:C_U1], AF.Exp)
    erho = work.tile([128, 2, M], F32, tag="erho", name="erho")
    nc.scalar.activation(erho[:], r[:], AF.Exp)
    evar = work.tile([128, 2, M], F32, tag="evar", name="evar")
    nc.scalar.activation(evar[:], P[:, :, C_V0:C_V1], AF.Exp)

    dsb = work.tile([128, 2, 3], F32, tag="dsb", name="dsb")
    nc.vector.tensor_copy(dsb[:], P[:, :, C_D0:C_D0 + 3])

    # pi chain first on DVE
    ssum = work.tile([128, 2, 1], F32, tag="ssum", name="ssum")
    nc.vector.tensor_reduce(ssum[:], epi_t[:], AX.X, OP.add)
    rsum = work.tile([128, 2, 1], F32, tag="rsum", name="rsum")
    nc.vector.reciprocal(rsum[:], ssum[:])
    for t in range(2):
        nc.vector.tensor_scalar_mul(out_p[:, 0, t, :], epi_t[:, t, :],
                                    rsum[:, t, 0:1])
    if OUT_HWDGE:
        nc.sync.dma_start(outpi_d.rearrange("t p o j -> p o t j"), out_p[:])
    else:
        nc.gpsimd.trigger_dma(count=1, signals_writable=[out_p[:]])

    # var chain
    z = work.tile([128, 2, M], F32, tag="z", name="z")
    nc.vector.tensor_scalar(z[:], erho[:], -1.0, 1.0, OP.mult, OP.add)
    for t in range(2):
        nc.vector.memset(z[:, t, 0:1], float(1.0 - np.exp(RHO_1)))
        nc.vector.memset(r[:, t, 0:1], float(RHO_1))
    nc.vector.tensor_tensor(out_v[:, 0, :, :], z[:], evar[:], OP.mult)
    if OUT_HWDGE:
        nc.scalar.dma_start(outvar_d.rearrange("t p o j -> p o t j"), out_v[:])
    else:
        nc.gpsimd.trigger_dma(count=1, signals_writable=[out_v[:]])

    # mu chain
    s = work.tile([128, 2, M], F32, tag="s", name="s")
    nc.vector.scalar_tensor_tensor(s[:], r[:], 1.0, eneg[:], OP.add, OP.mult)
    for t in range(2):
        nc.vector.memset(s[:, t, 0:1], S0)
    ss = work.tile([128, 2, M], F32, tag="ss", name="ss")
    nc.vector.tensor_tensor(ss[:], s[:], s[:], OP.mult)
    q = work.tile([128, 2, M], F32, tag="q", name="q")
    for t in range(2):
        nc.scalar.activation(q[:, t, :], s[:, t, :], AF.Identity,
                             bias=dsb[:, t, 0:1], scale=dsb[:, t, 1:2])
    rq = work.tile([128, 2, M], F32, tag="rq", name="rq")
    for t in range(2):
        nc.vector.tensor_tensor(rq[:, t, :], r[:, t, :], q[:, t, :], OP.mult)
        nc.vector.scalar_tensor_tensor(out_mu[:, 0, t, :], ss[:, t, :],
                                       dsb[:, t, 2:3], rq[:, t, :],
                                       OP.mult, OP.add)
    if OUT_HWDGE:
        nc.sync.dma_start(outmu_d.rearrange("t p o j -> p o t j"), out_mu[:])
    else:
        nc.gpsimd.trigger_dma(count=1, signals_writable=[out_mu[:]])


def _declare_io(nc):
    blk_d = nc.dram_tensor("blk", [128, KC, 2 * 128 + NW], F16,
                           kind="ExternalInput").ap()
    bias_d = nc.dram_tensor("bias", [1, 128 + NW], F16,
                            kind="ExternalInput").ap()
    outmu_d = nc.dram_tensor("out_mu", [2, 128, 1, M], F16,
                             kind="ExternalOutput").ap()
    outvar_d = nc.dram_tensor("out_var", [2, 128, 1, M], F16,
                              kind="ExternalOutput").ap()
    outpi_d = nc.dram_tensor("out_pi", [2, 128, 1, M], F16,
                             kind="ExternalOutput").ap()
    return blk_d, bias_d, outmu_d, outvar_d, outpi_d


def _warmup_act(nc, consts):
    # Load the Tanh/Exp ACT table immediately (one-time ~1.3us), and
    # init the PE filler source tile.
    warm_in = consts.tile([128, 1], F32, tag="warm_in", name="warm_in")
    warm_out = consts.tile([128, 1], F32, tag="warm_out", name="warm_out")
    nc.vector.memset(warm_in[:], 0.0)
    nc.scalar.activation(warm_out[:], warm_in[:], AF.Exp)
    warm = consts.tile([128, FILLER_COLS], F16, tag="pe_w", name="pe_w")
    nc.vector.memset(warm[:], 0.001)
    return warm


def _build(reps=None):
    nc = bacc.Bacc("TRN2", target_bir_lowering=False, debug=False)
    dram = _declare_io(nc)
    with tile.TileContext(nc) as tc, ExitStack() as ctx:
        consts = ctx.enter_context(tc.tile_pool(name="consts", bufs=1))
        blkpool = ctx.enter_context(tc.tile_pool(name="blk", bufs=1))
        psum = ctx.enter_context(tc.tile_pool(name="psum", bufs=1, space="PSUM"))
        work = ctx.enter_context(tc.tile_pool(name="work", bufs=1))
        pools = (consts, blkpool, psum, work)
        warm = _warmup_act(nc, consts)
        if reps is None:
            _emit_body(nc, tc, pools, dram, warm)
        else:
            with tc.For_i(0, reps, 1):
                _emit_body(nc, tc, pools, dram, warm)
    nc.compile()
    return nc


def build_loop_nc(reps):
    return _build(reps=reps)


_CACHE = {}


def _get_nc():
    if "nc" not in _CACHE:
        _CACHE["nc"] = _build()
    return _CACHE["nc"]


def _host_prep(inputs):
    f32 = np.float32
    feature = np.ascontiguousarray(inputs["feature"], dtype=f32)
    muW = np.asarray(inputs["muW"], dtype=f32)
    W = np.asarray(inputs["W"], dtype=f32)
    Z = np.asarray(inputs["Z"], dtype=f32)
    logvarW = np.asarray(inputs["logvarW"], dtype=f32)
    logvarZ = np.asarray(inputs["logvarZ"], dtype=f32)

    wstd = np.sqrt(np.exp(logvarW)).astype(f32)
    zstd = np.sqrt(np.exp(logvarZ)).astype(f32)
    a = ((zstd / wstd).astype(f32) * (W - muW)).astype(f32)
    v3 = np.stack([muW, a, Z, np.zeros_like(muW)], axis=1)      # [F, 4]

    wcat = np.concatenate(
        [-np.asarray(inputs["h2rho_w"], dtype=f32).T, v3,
         np.asarray(inputs["h2pi_w"], dtype=f32).T,
         np.asarray(inputs["h2var_w"], dtype=f32).T], axis=1)    # [F, 388]
    wcat = wcat.reshape(KC, 128, NW).astype(MM_NP)

    bias = np.concatenate(
        [np.ones(128, dtype=f32),
         -np.asarray(inputs["h2rho_b"], dtype=f32), np.zeros(4, dtype=f32),
         np.asarray(inputs["h2pi_b"], dtype=f32),
         np.asarray(inputs["h2var_b"], dtype=f32)]
    ).reshape(1, 128 + NW).astype(MM_NP)

    in_maps = []
    for cidx in range(NCORES):
        shard = feature[cidx * BC:(cidx + 1) * BC]       # [256, F]
        featT = shard.T.reshape(KC, 128, BC).astype(MM_NP)
        blk = np.empty((128, KC, 2 * 128 + NW), dtype=MM_NP)
        blk[:, :, 0:256] = featT.transpose(1, 0, 2)
        blk[:, :, 256:256 + NW] = wcat.transpose(1, 0, 2)
        in_maps.append({"blk": np.ascontiguousarray(blk), "bias": bias})
    return in_maps


def kernel(**inputs):
    nc = _get_nc()
    in_maps = _host_prep(inputs)
    res = run_bass_kernel_spmd(nc, in_maps, list(range(NCORES)))
    mu = np.concatenate(
        [res.results[c]["out_mu"].reshape(BC, M) for c in range(NCORES)],
        axis=0).astype(np.float32)
    var = np.concatenate(
        [res.results[c]["out_var"].reshape(BC, M) for c in range(NCORES)],
        axis=0).astype(np.float32)
    pi = np.concatenate(
        [res.results[c]["out_pi"].reshape(BC, M) for c in range(NCORES)],
        axis=0).astype(np.float32)
    return pi, mu, var
